# revision 59
# baseline (speedup 1.0000x reference)
"""Trainium2 Bass kernel for the attention module:

    xp      = x @ W.T + b                      # [B, E]
    scores  = einsum('be,tbe->bt', xp, enc)    # [B, T]
    attn    = softmax(scores, axis=1)
    context = einsum('bt,tbe->be', attn, enc)  # [B, E]
    out     = concat([xp, context], axis=1)    # [B, 2E]

Shapes: T=2048, B=128, D_dec=512, E=512 (fp32).

Strategy (data-parallel over batch, 8 NeuronCores, no collectives):
  - Each core owns NB=16 batches: its encoder_states shard is
    [T, 16, E] = 64 MiB fp32, streamed from HBM exactly once in NT=16
    t-tiles of [128, 16, 512], CAST TO FP16 during the SWDGE DMA.
    The SWDGE queue carries ONLY the enc stream; everything else
    (xT/WT/bias loads, output stores) rides HWDGE (nc.sync), so the
    stream starts at t~8us and never yields the queue.
  - xp is computed with 4 fp16 accumulating matmuls directly in the
    natural [b, e] layout (stationary = x^T chunk, moving = W^T chunk),
    bias comes pre-replicated from the host, and the all-partition
    broadcast xpb is built by GpSimd partition_broadcast (SBUF->SBUF,
    no DRAM bounce, no DMA-queue time).
  - Scores S[t,b] = sum_e enc*xp are split across three engines per
    tile so no engine exceeds the DMA period:
      batches  0: 6  DVE multiply (fp16 2x) -> ScalarE activation-accum
      batches  6:12  DVE multiply (fp16 2x) -> GpSimd tensor_reduce
      batches 12:16  DVE fused tensor_tensor_reduce (1x) direct to S
  - Per-tile tail (softmax + context accumulation) is unchanged from
    the flash-style deferred-combine scheme: TensorE transposes, exp
    on ScalarE with accumulated l_k, and 16 masked fp16 matmuls into a
    PSUM context tile; tails run two tiles behind heads.
  - Final exact softmax combine over the 16 (m_k, l_k, c_k): weight
    math on DVE, the 16-term weighted sum split between DVE (2
    interleaved chains, tiles 0..7) and GpSimd (2 chains, 8..14),
    merged on DVE.

This toolchain's walrus accepts AT MOST ONE semaphore wait per TPB
compute instruction, and Tile pool slot reuse emits extra release
waits.  Hence: hot buffers are allocated once and alternated manually,
and cheap "observer" ops make each engine see a new producer before
the real consumer runs, keeping every instruction at <= 1 wait.
"""

import os
import sys

import numpy as np

if "/opt/trn_rl_repo" not in sys.path and not any(
    os.path.isdir(os.path.join(p, "concourse")) for p in sys.path if p
):
    sys.path.insert(0, "/opt/trn_rl_repo")

import concourse.bass as bass
import concourse.mybir as mybir
import concourse.tile as tile
from concourse.bass_utils import run_bass_kernel_spmd
from concourse.masks import make_identity
from concourse.tile_rust import add_dep_helper

T, B, D, E = 2048, 128, 512, 512
NCORES = 8
NB = B // NCORES  # 16 local batches per core
PT = 128          # t-tile partition size
NT = T // PT      # 16 t-tiles
NC_D = D // 128   # 4 chunks of the contraction dim for the xp matmul
NBUF = 6          # rotating fp16 enc tile buffers
LAG = 1           # tail(k-LAG) is emitted after head(k)

# score-reduction engine split: batches [0:NA) -> ScalarE activation-
# accumulate, [NA:16) -> one batched DVE tensor_reduce.
NA = 8
NR = NB - NA
NTT2 = 12          # last-tile DMA split point (batches 0:12 / 12:16)

F32 = mybir.dt.float32
F16 = mybir.dt.float16
AF = mybir.ActivationFunctionType
ALU = mybir.AluOpType
AX = mybir.AxisListType


def _install_drain_split():
    """This walrus rejects instructions carrying more than one semaphore
    wait.  Tile's kernel-tail drain waits on every proc's final tick in a
    single instruction; split it into one drain per wait."""
    from concourse.vector_clock import ScopedClock

    if getattr(tile.TileContext, "_drain_split_installed", False):
        return

    def _split_dab(self, tick_clock, wait_clock):
        drain_inst = self.nc.sync.drain()
        wait_clock.add_sem_waits(
            drain_inst.ins, ScopedClock({None: tick_clock.global_clock})
        )
        si = drain_inst.ins.sync_info
        if si is not None and len(si.on_wait) > 1:
            waits = list(si.on_wait)
            upds = list(si.on_update)
            drain_inst.ins.sync_info = mybir.SyncInfo(
                on_wait=[waits[0]], on_update=upds
            )
            for w in waits[1:]:
                d2 = self.nc.sync.drain()
                d2.ins.sync_info = mybir.SyncInfo(on_wait=[w], on_update=[])

        self.nc.all_engine_barrier()
        assert self.sems is not None
        popped = self.nc._tile_sem_poison_stack.pop()
        assert popped is self._sem_poison
        self.nc.clear_and_free_semaphores(list(self.sems.allocated().values()))
        self.nc.all_engine_barrier()

    tile.TileContext._drain_and_barrier = _split_dab
    tile.TileContext._drain_split_installed = True


_install_drain_split()


def build_nc() -> bass.Bass:
    nc = bass.Bass()

    # Per-core shards (host pre-swizzles / pre-casts the small operands so
    # every DMA partition line is one contiguous descriptor).
    xT_ext = nc.declare_dram_parameter("xT", [128, NC_D * NB], F16, isOutput=False)
    WT_ext = nc.declare_dram_parameter("WT", [128, NC_D * E], F16, isOutput=False)
    b_ext = nc.declare_dram_parameter("bias", [NB, E], F32, isOutput=False)
    enc_ext = nc.declare_dram_parameter("enc", [T, NB, E], F32, isOutput=False)
    out_ext = nc.declare_dram_parameter("out", [NB, 2 * E], F32, isOutput=True)

    with tile.TileContext(nc) as tc:
        with (
            tc.tile_pool(name="sb", bufs=1) as sb,
            tc.tile_pool(name="dram", bufs=1, space="DRAM") as dram_pool,
            tc.tile_pool(name="ps", bufs=1, space="PSUM") as ps,
        ):
            # ---- HWDGE: small operands -------------------------------------
            xT_sb = sb.tile([128, NC_D, NB], F16)
            nc.sync.dma_start(out=xT_sb[:], in_=xT_ext[:, :])
            WT_sb = sb.tile([128, NC_D, E], F16)
            nc.sync.dma_start(out=WT_sb[:], in_=WT_ext[:, :])
            b_sb = sb.tile([NB, E], F32)
            nc.sync.dma_start(out=b_sb[:], in_=b_ext[:, :])

            # identities (GpSimd writes; PE observes before first use).
            # Emitted before enc0 so the Pool program is: identities,
            # enc0, xpb broadcast (stalls on the bounce), enc1..NBUF-1.
            ident = sb.tile([128, 128], F32)
            make_identity(nc, ident[:])
            identH = sb.tile([128, 128], F16)
            make_identity(nc, identH[:])

            # PSUM tiles: 8 banks total
            obs_ps = ps.tile([1, 16], F32, tag="obs")
            xp_ps = ps.tile([NB, E], F32, tag="xp")
            sT_ps = ps.tile([NB, PT], F32, tag="sT")
            p_ps = ps.tile([PT, NB], F16, tag="p")
            ctx_pair = [ps.tile([NB, E], F32, name=f"ctx{i}") for i in range(2)]
            bc_ps = [ps.tile([128, E], F32, name=f"bc{i}") for i in range(2)]

            def pe_observe(ap, obs):
                return nc.tensor.matmul(obs[:], lhsT=ap[:, 0:1], rhs=ap[:, 0:16],
                                        start=True, stop=True)

            pe_observe(ident, obs_ps)   # PE observes the fp32 identity
            pe_observe(identH, obs_ps)  # PE observes the fp16 identity

            # ---- xp = x @ W.T + b, directly in natural [b, e] layout -------
            # stationary = xT chunk [128(d), 16(b)], moving = WT chunk
            # [128(d), 512(e)]; accumulate the 4 d-chunks into one PSUM bank.
            obs_xt = pe_observe(xT_sb[:, 0, :], obs_ps)  # PE observes xT DMA
            for cd in range(NC_D):
                mm = nc.tensor.matmul(
                    xp_ps[:],
                    lhsT=xT_sb[:, cd, :],
                    rhs=WT_sb[:, cd, :],
                    start=(cd == 0),
                    stop=(cd == NC_D - 1),
                )
                if cd == 0:
                    add_dep_helper(mm.ins, obs_xt.ins, sync=False)

            out_tile = sb.tile([NB, 2 * E], F32)
            junk_b = sb.tile([NB, 1], F32)
            nc.vector.tensor_copy(junk_b[:], b_sb[:, 0:1])  # DVE observes b DMA
            nc.vector.tensor_tensor(
                out=out_tile[:, 0:E], in0=xp_ps[:], in1=b_sb[:], op=ALU.add
            )
            xp16 = sb.tile([NB, E], F16)
            nc.vector.tensor_copy(xp16[:], out_tile[:, 0:E])

            # xp half of the output never changes; ship it now on HWDGE.
            nc.sync.dma_start(out=out_ext[:, 0:E], in_=out_tile[:, 0:E])

            # Broadcast xp to all 128 partitions ON-CHIP: a tiny HWDGE
            # SBUF->SBUF DMA flattens xp16 onto partition 0 (PE moving
            # operands must start at partition 0); then for each batch b,
            # one matmul with stationary ones[1, 128] and moving
            # xp_flat[b*E:(b+1)*E] replicates the row into a PSUM bank;
            # DVE and ScalarE alternate copying the banks out to xpb.
            # No HBM traffic, no DMA-queue time, ready alongside enc0.
            xp_flat = sb.tile([1, NB * E], F16)
            nc.sync.dma_start(
                out=xp_flat[0:1, :].rearrange("o (b e) -> o b e", b=NB),
                in_=xp16[:],
            )
            xpb = sb.tile([128, NB, E], F16)
            ones1 = sb.tile([1, 128], F16)
            nc.vector.memset(ones1[:], 1.0)
            prev_mm = None
            for b in range(NB):
                mm = nc.tensor.matmul(
                    bc_ps[b % 2][:], lhsT=ones1[0:1, :],
                    rhs=xp_flat[0:1, b * E : (b + 1) * E],
                    start=True, stop=True,
                )
                if prev_mm is not None:
                    add_dep_helper(mm.ins, prev_mm.ins, sync=False)
                prev_mm = mm
                if b % 2 == 0:
                    nc.vector.tensor_copy(xpb[:, b, :], bc_ps[0][:])
                else:
                    nc.scalar.activation(xpb[:, b, :], bc_ps[1][:], AF.Copy)

            # ---- SWDGE queue: pure enc stream ------------------------------
            enc_t = [sb.tile([PT, NB, E], F16, name=f"enc{i}") for i in range(NBUF)]
            dmas = {}
            for k in range(NBUF):
                dmas[k] = nc.gpsimd.dma_start(
                    out=enc_t[k][:], in_=enc_ext[k * PT : (k + 1) * PT, :, :]
                )

            # ---- persistent buffers for the t-tile loop -------------------
            # 2 rotating prod buffers; the jes/j2 observers carry all DMA
            # semaphores, so each multiply's only wait is its posted-write
            # hazard.
            prod = [sb.tile([PT, NB, E], F16, name=f"prod{i}") for i in range(2)]
            S_all = sb.tile([PT, NT, NB], F32)
            pT_all = sb.tile([NB, NT, PT], F16)
            pm_t = [sb.tile([PT, NB, NB], F16, name=f"pm{i}") for i in range(4)]
            for t in pm_t:
                nc.vector.memset(t[:], 0.0)  # off-diagonals stay 0 forever
            junk_ss = sb.tile([NB, NT], F32)
            dummy_all = sb.tile([PT, NT, NA], F32)  # write-once accum dummies
            jw = sb.tile([1, 8], F32)
            jns = sb.tile([NB, NT], F32)
            jha = sb.tile([NB, 2], F32)
            jese = sb.tile([1, NT], F32)
            jes2 = sb.tile([1, 1], F32)

            NEGM = sb.tile([NB, NT], F32)   # -m_k per (b, k)
            L_all = sb.tile([NB, NT], F32)  # l_k per (b, k)
            c_store = sb.tile([NB, NT, E], F16)

            # ---- software-pipelined t-tile loop --------------------------
            hist = {}
            tts = {}
            cstore_q = []  # tiles whose ctx_ps copy is still pending

            def emit_tail(k):
                eth = enc_t[k % NBUF]
                ctx_ps = ctx_pair[k % 2]
                jst = None
                if k == 0:
                    # PE observes tile 0's ScalarE-written S columns so the
                    # first transpose carries only the DVE wait
                    jst = nc.tensor.matmul(
                        obs_ps[:, 0:NA],
                        lhsT=S_all[:, 0, 0:1], rhs=S_all[:, 0, 0:NA],
                        start=True, stop=True,
                    )
                st_inst = nc.tensor.transpose(sT_ps[:], S_all[:, k, :], ident[0:PT, 0:PT])
                if jst is not None:
                    add_dep_helper(st_inst.ins, jst.ins, sync=False)
                if cstore_q:
                    # the pending c_store copy must land before this tile's
                    # ctx matmuls reuse ctx_ps; syncing the S transpose on it
                    # both orders the streams and keeps jmm at one wait
                    add_dep_helper(st_inst.ins, cstore_q[-1].ins, sync=True)
                if k >= 1:
                    # DVE observes exp(k-1) (the last sT_ps reader) so the
                    # NEGM reduce carries only the PE wait
                    nc.vector.tensor_copy(jns[:, k : k + 1], pT_all[:, k - 1, 0:1])
                nc.vector.tensor_reduce(
                    out=NEGM[:, k : k + 1], in_=sT_ps[:], axis=AX.X, op=ALU.max,
                    negate=True,
                )
                nc.scalar.activation(junk_ss[:, k : k + 1], sT_ps[:, 0:1], AF.Copy)
                nc.scalar.activation(
                    pT_all[:, k, :], sT_ps[:], AF.Exp,
                    bias=NEGM[:, k : k + 1], scale=1.0,
                    accum_out=L_all[:, k : k + 1],
                )
                nc.tensor.transpose(p_ps[:], pT_all[:, k, :], identH[0:NB, 0:NB])
                pm = pm_t[k % 4]
                nc.vector.tensor_copy(
                    pm[:, :, :].rearrange("p a b -> p (a b)")[:, :: NB + 1],
                    p_ps[:],
                )
                # PE observes the enc DMA(s) before the ctx matmuls use it
                jmm = nc.tensor.matmul(
                    ctx_ps[0:1, 0:16],
                    lhsT=eth[:, 0, 0:1], rhs=eth[:, 0, 0:16],
                    start=True, stop=True,
                )
                add_dep_helper(jmm.ins, st_inst.ins, sync=False)
                if isinstance(dmas[k], tuple):
                    jmm2 = nc.tensor.matmul(
                        ctx_ps[0:1, 0:16],
                        lhsT=eth[:, NTT2, 0:1], rhs=eth[:, NTT2, 0:16],
                        start=True, stop=True,
                    )
                    add_dep_helper(jmm2.ins, jmm.ins, sync=False)
                last_mm = None
                for b in range(NB):
                    last_mm = nc.tensor.matmul(
                        ctx_ps[:],
                        lhsT=pm[:, b, :],
                        rhs=eth[:, b, :],
                        start=(b == 0),
                        stop=(b == NB - 1),
                    )
                hist[k % NBUF] = (last_mm, tts[k], dmas[k][0] if isinstance(dmas[k], tuple) else dmas[k])
                return last_mm

            def emit_cstore(k):
                inst = nc.scalar.activation(
                    c_store[:, k, :], ctx_pair[k % 2][:], AF.Copy
                )
                cstore_q.append(inst)
                return inst

            for k in range(NT):
                eth = enc_t[k % NBUF]
                pr = prod[k % 2]
                # Pre-absorb the buffer's reuse hazards on the Pool proc
                # with explicitly-synced nops, so the SWDGE DMA needs no
                # more than the allowed number of waits.
                if k >= NBUF:
                    nop = None
                    for dep in hist[k % NBUF]:
                        prev_nop = nop
                        nop = nc.gpsimd.engine_nop()
                        add_dep_helper(nop.ins, dep.ins, sync=True)
                        if prev_nop is not None:
                            add_dep_helper(nop.ins, prev_nop.ins, sync=False)
                    if k == NT - 1:
                        # The final tile arrives as two batch-halves so its
                        # fp16 multiplies overlap the second half's transfer.
                        da = nc.gpsimd.dma_start(
                            out=eth[:, 0:NTT2, :],
                            in_=enc_ext[k * PT : (k + 1) * PT, 0:NTT2, :],
                        )
                        add_dep_helper(da.ins, nop.ins, sync=False)
                        db = nc.gpsimd.dma_start(
                            out=eth[:, NTT2:NB, :],
                            in_=enc_ext[k * PT : (k + 1) * PT, NTT2:NB, :],
                        )
                        add_dep_helper(db.ins, da.ins, sync=False)
                        dmas[k] = (da, db)
                    else:
                        dmas[k] = nc.gpsimd.dma_start(
                            out=eth[:], in_=enc_ext[k * PT : (k + 1) * PT, :, :]
                        )

                # --- DVE head -------------------------------------------
                jwl = None
                if k == 0:
                    # absorb the xpb waits for tt_a's batch range [0:NA):
                    # one observer for the ScalarE-copied odd batches, one
                    # for the DVE posted-write ack of the even batches
                    jw1 = nc.vector.tensor_copy(jw[0:1, 0:1], xpb[0:1, NA - 1, 0:1])
                    jwl = nc.vector.tensor_copy(jw[0:1, 1:2], xpb[0:1, NA - 2, 0:1])
                    add_dep_helper(jwl.ins, jw1.ins, sync=False)
                if k in (2, 3):
                    # ScalarE's reads of prod[k%2] (accums k-2) are not yet
                    # covered by the exp observer chain; absorb explicitly.
                    nc.vector.tensor_copy(
                        jha[:, k - 2 : k - 1], S_all[0:NB, k - 2, NA - 1 : NA]
                    )
                # A dedicated observer carries this tile's enc DMA wait so
                # the multiplies are free to carry their prod posted-write
                # self-waits.  Orders are pinned so the scheduler cannot
                # move the DMA wait onto a multiply.
                jes = nc.vector.tensor_copy(jese[0:1, k : k + 1], eth[0:1, 0, 0:1])
                if jwl is not None:
                    add_dep_helper(jes.ins, jwl.ins, sync=False)
                tt_a = nc.vector.tensor_tensor(
                    out=pr[:, 0:NA, :], in0=eth[:, 0:NA, :],
                    in1=xpb[:, 0:NA, :], op=ALU.mult,
                )
                add_dep_helper(tt_a.ins, jes.ins, sync=False)
                if k == NT - 1:
                    # split last tile: multiply the first-half batches, then
                    # observe the second-half DMA, then multiply the rest
                    tt_b1 = nc.vector.tensor_tensor(
                        out=pr[:, NA:NTT2, :], in0=eth[:, NA:NTT2, :],
                        in1=xpb[:, NA:NTT2, :], op=ALU.mult,
                    )
                    add_dep_helper(tt_b1.ins, tt_a.ins, sync=False)
                    j2 = nc.vector.tensor_copy(jes2[:], eth[0:1, NB - 1, 0:1])
                    add_dep_helper(j2.ins, tt_b1.ins, sync=False)
                    tt_b = nc.vector.tensor_tensor(
                        out=pr[:, NTT2:NB, :], in0=eth[:, NTT2:NB, :],
                        in1=xpb[:, NTT2:NB, :], op=ALU.mult,
                    )
                    add_dep_helper(tt_b.ins, j2.ins, sync=False)
                else:
                    prev = tt_a
                    if k == 0:
                        jw3 = nc.vector.tensor_copy(jw[0:1, 2:3], xpb[0:1, NB - 1, 0:1])
                        add_dep_helper(jw3.ins, prev.ins, sync=False)
                        jw4 = nc.vector.tensor_copy(jw[0:1, 3:4], xpb[0:1, NB - 2, 0:1])
                        add_dep_helper(jw4.ins, jw3.ins, sync=False)
                        prev = jw4
                    tt_b = nc.vector.tensor_tensor(
                        out=pr[:, NA:NB, :], in0=eth[:, NA:NB, :],
                        in1=xpb[:, NA:NB, :], op=ALU.mult,
                    )
                    add_dep_helper(tt_b.ins, prev.ins, sync=False)
                # one batched reduce produces the DVE half of the S column
                red = nc.vector.tensor_reduce(
                    out=S_all[:, k, NA:NB], in_=pr[:, NA:NB, :], axis=AX.X,
                    op=ALU.add,
                )
                add_dep_helper(red.ins, tt_b.ins, sync=False)
                tts[k] = red

                # --- ScalarE head: activation-accumulate batches 0:NA ----
                for i, b in enumerate(range(0, NA)):
                    nc.scalar.activation(
                        dummy_all[:, k, i : i + 1].broadcast_to((PT, E)),
                        pr[:, b, :],
                        AF.Copy,
                        accum_out=S_all[:, k, b : b + 1],
                    )
                    if i == 0 and k >= LAG + 1:
                        emit_cstore(k - LAG - 1)

                if k >= LAG:
                    emit_tail(k - LAG)

            emit_tail(NT - 1)
            emit_cstore(NT - 2)

            # ---- final combine across tiles -------------------------------
            # Weight math on DVE, ordered after c_store(14) so the chain
            # ACT waits for tiles <= 14 all elide.  The 16-term weighted
            # sum runs as 2 DVE chains (tiles 0..7) + 2 GpSimd chains
            # (8..14), interleaved so posted-write self-waits hide, with
            # tile 15's term appended on DVE after its c_store lands.
            negM = sb.tile([NB, 1], F32)
            nc.vector.tensor_reduce(out=negM[:], in_=NEGM[:], axis=AX.X, op=ALU.min)
            alpha = sb.tile([NB, NT], F32)
            ainst = nc.scalar.activation(
                alpha[:], NEGM[:], AF.Exp, bias=negM[:], scale=-1.0
            )
            add_dep_helper(ainst.ins, cstore_q[-1].ins, sync=False)
            prodw = sb.tile([NB, NT], F32)
            nc.vector.tensor_tensor(out=prodw[:], in0=alpha[:], in1=L_all[:],
                                    op=ALU.mult)
            Lsum = sb.tile([NB, 1], F32)
            nc.vector.tensor_reduce(out=Lsum[:], in_=prodw[:], axis=AX.X, op=ALU.add)
            rL = sb.tile([NB, 1], F32)
            nc.vector.reciprocal(rL[:], Lsum[:])
            w = sb.tile([NB, NT], F32)
            nc.vector.tensor_scalar_mul(w[:], alpha[:], rL[:])

            # Weighted sum of the 16 stored contexts on DVE as TWO
            # interleaved accumulation chains, so each op's posted-write
            # self-wait hides under the other chain's execution.
            acc = out_tile[:, E : 2 * E]
            acc_b = sb.tile([NB, E], F32)
            nc.vector.tensor_copy(jw[0:1, 4:5], w[0:1, 0:1])  # absorb w's self-wait
            nc.vector.tensor_scalar_mul(acc, c_store[:, 0, :], w[:, 0:1])
            h = NT // 2
            nc.vector.tensor_scalar_mul(acc_b[:], c_store[:, h, :], w[:, h : h + 1])
            for k in range(1, h):
                nc.vector.scalar_tensor_tensor(
                    out=acc, in0=c_store[:, k, :], scalar=w[:, k : k + 1], in1=acc,
                    op0=ALU.mult, op1=ALU.add,
                )
                if h + k < NT - 1:
                    nc.vector.scalar_tensor_tensor(
                        out=acc_b[:], in0=c_store[:, h + k, :],
                        scalar=w[:, h + k : h + k + 1], in1=acc_b[:],
                        op0=ALU.mult, op1=ALU.add,
                    )

            emit_cstore(NT - 1)
            # DVE observes the last c_store copy, then finishes the chain
            nc.vector.tensor_copy(jw[0:1, 5:6], c_store[0:1, NT - 1, 0:1])
            nc.vector.scalar_tensor_tensor(
                out=acc_b[:], in0=c_store[:, NT - 1, :],
                scalar=w[:, NT - 1 : NT], in1=acc_b[:],
                op0=ALU.mult, op1=ALU.add,
            )
            nc.vector.tensor_tensor(out=acc, in0=acc, in1=acc_b[:], op=ALU.add)

            nc.sync.dma_start(out=out_ext[:, E : 2 * E], in_=acc)

    # Raw Bass does not lower InstISA subclasses (tensor_tensor_reduce);
    # without this the NEFF compiler sees empty .instr -> "ISA wrong length".
    mybir.codegen_inst_isa_subclasses(nc)
    return nc


_NC_CACHE: bass.Bass | None = None


def _get_nc() -> bass.Bass:
    global _NC_CACHE
    if _NC_CACHE is None:
        _NC_CACHE = build_nc()
    return _NC_CACHE


def make_in_maps(inputs: dict) -> list[dict]:
    x = np.asarray(inputs["x"], dtype=np.float32)
    enc = np.asarray(inputs["encoder_states"], dtype=np.float32)
    W = np.asarray(inputs["W"], dtype=np.float32)
    bias = np.asarray(inputs["b"], dtype=np.float32)

    # partition-major swizzle: element (p, c*X + j) = src[c*128 + p, j],
    # so each SBUF partition line is one contiguous DMA descriptor.
    WT16 = np.ascontiguousarray(
        W.T.astype(np.float16).reshape(NC_D, 128, E).transpose(1, 0, 2).reshape(
            128, NC_D * E
        )
    )
    b_rep = np.ascontiguousarray(np.broadcast_to(bias[None, :], (NB, E))).astype(
        np.float32
    )
    in_maps = []
    for i in range(NCORES):
        sl = slice(i * NB, (i + 1) * NB)
        xT16 = np.ascontiguousarray(
            x[sl].T.astype(np.float16)
            .reshape(NC_D, 128, NB).transpose(1, 0, 2).reshape(128, NC_D * NB)
        )
        in_maps.append(
            {
                "xT": xT16,
                "WT": WT16,
                "bias": b_rep,
                "enc": np.ascontiguousarray(enc[:, sl, :]),
            }
        )
    return in_maps


def run(inputs: dict, trace: bool = False, tmpdir: str | None = None):
    """Returns (full_output [B, 2E] f32, exec_time_ns or None)."""
    nc = _get_nc()
    in_maps = make_in_maps(inputs)
    res = run_bass_kernel_spmd(
        nc, in_maps, core_ids=list(range(NCORES)), trace=trace, tmpdir=tmpdir
    )
    out = np.concatenate([res.results[i]["out"] for i in range(NCORES)], axis=0)
    return out.astype(np.float32), res.exec_time_ns


def kernel(**inputs) -> np.ndarray:
    out, _ = run(inputs, trace=False)
    return out


# revision 60
# speedup vs baseline: 1.0358x; 1.0358x over previous
"""Trainium2 Bass kernel for the attention module:

    xp      = x @ W.T + b                      # [B, E]
    scores  = einsum('be,tbe->bt', xp, enc)    # [B, T]
    attn    = softmax(scores, axis=1)
    context = einsum('bt,tbe->be', attn, enc)  # [B, E]
    out     = concat([xp, context], axis=1)    # [B, 2E]

Shapes: T=2048, B=128, D_dec=512, E=512 (fp32).

Strategy (data-parallel over batch, 8 NeuronCores, no collectives):
  - Each core owns NB=16 batches: its encoder_states shard is
    [T, 16, E] = 64 MiB fp32, streamed from HBM exactly once in NT=16
    t-tiles of [128, 16, 512], CAST TO FP16 during the SWDGE DMA.
    The SWDGE queue carries ONLY the enc stream; everything else
    (xT/WT/bias loads, output stores) rides HWDGE (nc.sync), so the
    stream starts at t~8us and never yields the queue.
  - xp is computed with 4 fp16 accumulating matmuls directly in the
    natural [b, e] layout (stationary = x^T chunk, moving = W^T chunk),
    bias comes pre-replicated from the host, and the all-partition
    broadcast xpb is built by GpSimd partition_broadcast (SBUF->SBUF,
    no DRAM bounce, no DMA-queue time).
  - Scores S[t,b] = sum_e enc*xp are split across three engines per
    tile so no engine exceeds the DMA period:
      batches  0: 6  DVE multiply (fp16 2x) -> ScalarE activation-accum
      batches  6:12  DVE multiply (fp16 2x) -> GpSimd tensor_reduce
      batches 12:16  DVE fused tensor_tensor_reduce (1x) direct to S
  - Per-tile tail (softmax + context accumulation) is unchanged from
    the flash-style deferred-combine scheme: TensorE transposes, exp
    on ScalarE with accumulated l_k, and 16 masked fp16 matmuls into a
    PSUM context tile; tails run two tiles behind heads.
  - Final exact softmax combine over the 16 (m_k, l_k, c_k): weight
    math on DVE, the 16-term weighted sum split between DVE (2
    interleaved chains, tiles 0..7) and GpSimd (2 chains, 8..14),
    merged on DVE.

This toolchain's walrus accepts AT MOST ONE semaphore wait per TPB
compute instruction, and Tile pool slot reuse emits extra release
waits.  Hence: hot buffers are allocated once and alternated manually,
and cheap "observer" ops make each engine see a new producer before
the real consumer runs, keeping every instruction at <= 1 wait.
"""

import os
import sys

import numpy as np

if "/opt/trn_rl_repo" not in sys.path and not any(
    os.path.isdir(os.path.join(p, "concourse")) for p in sys.path if p
):
    sys.path.insert(0, "/opt/trn_rl_repo")

import concourse.bass as bass
import concourse.mybir as mybir
import concourse.tile as tile
from concourse.bass_utils import run_bass_kernel_spmd
from concourse.masks import make_identity
from concourse.tile_rust import add_dep_helper

T, B, D, E = 2048, 128, 512, 512
NCORES = 8
NB = B // NCORES  # 16 local batches per core
PT = 128          # t-tile partition size
NT = T // PT      # 16 t-tiles
NC_D = D // 128   # 4 chunks of the contraction dim for the xp matmul
NBUF = 6          # rotating fp16 enc tile buffers
LAG = 1           # tail(k-LAG) is emitted after head(k)

# score-reduction engine split: batches [0:NA) -> ScalarE activation-
# accumulate, [NA:16) -> one batched DVE tensor_reduce.
NA = 8
NR = NB - NA
NTT2 = 12          # last-tile DMA split point (batches 0:12 / 12:16)

F32 = mybir.dt.float32
F16 = mybir.dt.float16
AF = mybir.ActivationFunctionType
ALU = mybir.AluOpType
AX = mybir.AxisListType


def _install_drain_split():
    """This walrus rejects instructions carrying more than one semaphore
    wait.  Tile's kernel-tail drain waits on every proc's final tick in a
    single instruction; split it into one drain per wait."""
    from concourse.vector_clock import ScopedClock

    if getattr(tile.TileContext, "_drain_split_installed", False):
        return

    def _split_dab(self, tick_clock, wait_clock):
        drain_inst = self.nc.sync.drain()
        wait_clock.add_sem_waits(
            drain_inst.ins, ScopedClock({None: tick_clock.global_clock})
        )
        si = drain_inst.ins.sync_info
        if si is not None and len(si.on_wait) > 1:
            waits = list(si.on_wait)
            upds = list(si.on_update)
            drain_inst.ins.sync_info = mybir.SyncInfo(
                on_wait=[waits[0]], on_update=upds
            )
            for w in waits[1:]:
                d2 = self.nc.sync.drain()
                d2.ins.sync_info = mybir.SyncInfo(on_wait=[w], on_update=[])

        self.nc.all_engine_barrier()
        assert self.sems is not None
        popped = self.nc._tile_sem_poison_stack.pop()
        assert popped is self._sem_poison
        self.nc.clear_and_free_semaphores(list(self.sems.allocated().values()))
        self.nc.all_engine_barrier()

    tile.TileContext._drain_and_barrier = _split_dab
    tile.TileContext._drain_split_installed = True


_install_drain_split()


def build_nc() -> bass.Bass:
    nc = bass.Bass()

    # Per-core shards (host pre-swizzles / pre-casts the small operands so
    # every DMA partition line is one contiguous descriptor).
    xT_ext = nc.declare_dram_parameter("xT", [128, NC_D * NB], F16, isOutput=False)
    WT_ext = nc.declare_dram_parameter("WT", [128, NC_D * E], F16, isOutput=False)
    b_ext = nc.declare_dram_parameter("bias", [NB, E], F32, isOutput=False)
    enc_ext = nc.declare_dram_parameter("enc", [T, NB, E], F32, isOutput=False)
    out_ext = nc.declare_dram_parameter("out", [NB, 2 * E], F32, isOutput=True)

    with tile.TileContext(nc) as tc:
        with (
            tc.tile_pool(name="sb", bufs=1) as sb,
            tc.tile_pool(name="dram", bufs=1, space="DRAM") as dram_pool,
            tc.tile_pool(name="ps", bufs=1, space="PSUM") as ps,
        ):
            # ---- HWDGE: small operands -------------------------------------
            xT_sb = sb.tile([128, NC_D, NB], F16)
            nc.sync.dma_start(out=xT_sb[:], in_=xT_ext[:, :])
            WT_sb = sb.tile([128, NC_D, E], F16)
            nc.sync.dma_start(out=WT_sb[:], in_=WT_ext[:, :])
            b_sb = sb.tile([NB, E], F32)
            nc.sync.dma_start(out=b_sb[:], in_=b_ext[:, :])

            # identities (GpSimd writes; PE observes before first use).
            # Emitted before enc0 so the Pool program is: identities,
            # enc0, xpb broadcast (stalls on the bounce), enc1..NBUF-1.
            ident = sb.tile([128, 128], F32)
            make_identity(nc, ident[:])
            identH = sb.tile([128, 128], F16)
            make_identity(nc, identH[:])

            # PSUM tiles: 8 banks total
            obs_ps = ps.tile([1, 16], F32, tag="obs")
            xp_ps = ps.tile([NB, E], F32, tag="xp")
            sT_ps = ps.tile([NB, PT], F32, tag="sT")
            p_ps = ps.tile([PT, NB], F16, tag="p")
            ctx_pair = [ps.tile([NB, E], F32, name=f"ctx{i}") for i in range(2)]

            def pe_observe(ap, obs):
                return nc.tensor.matmul(obs[:], lhsT=ap[:, 0:1], rhs=ap[:, 0:16],
                                        start=True, stop=True)

            pe_observe(ident, obs_ps)   # PE observes the fp32 identity
            pe_observe(identH, obs_ps)  # PE observes the fp16 identity

            # ---- xp = x @ W.T + b, directly in natural [b, e] layout -------
            # stationary = xT chunk [128(d), 16(b)], moving = WT chunk
            # [128(d), 512(e)]; accumulate the 4 d-chunks into one PSUM bank.
            obs_xt = pe_observe(xT_sb[:, 0, :], obs_ps)  # PE observes xT DMA
            for cd in range(NC_D):
                mm = nc.tensor.matmul(
                    xp_ps[:],
                    lhsT=xT_sb[:, cd, :],
                    rhs=WT_sb[:, cd, :],
                    start=(cd == 0),
                    stop=(cd == NC_D - 1),
                )
                if cd == 0:
                    add_dep_helper(mm.ins, obs_xt.ins, sync=False)

            out_tile = sb.tile([NB, 2 * E], F32)
            junk_b = sb.tile([NB, 1], F32)
            nc.vector.tensor_copy(junk_b[:], b_sb[:, 0:1])  # DVE observes b DMA
            nc.vector.tensor_tensor(
                out=out_tile[:, 0:E], in0=xp_ps[:], in1=b_sb[:], op=ALU.add
            )
            xp16 = sb.tile([NB, E], F16)
            nc.vector.tensor_copy(xp16[:], out_tile[:, 0:E])

            # xp half of the output never changes; ship it now on HWDGE.
            nc.sync.dma_start(out=out_ext[:, 0:E], in_=out_tile[:, 0:E])

            # Broadcast xp (fp16) to all 128 partitions via DRAM bounce:
            # HWDGE writes xp16 to DRAM; a SWDGE read with a 0-stride
            # partition dim replicates it.  The read is pinned between
            # enc0 and enc1 on the Pool queue so it lands early without
            # ever starving the stream (enc0's descriptors keep the SDMA
            # engines busy while Pool waits for the bounce).
            xp_dram = dram_pool.tile([NB, E], F16)
            nc.sync.dma_start(out=xp_dram[:], in_=xp16[:])
            xpb = sb.tile([128, NB, E], F16)

            # ---- SWDGE queue: pure enc stream ------------------------------
            enc_t = [sb.tile([PT, NB, E], F16, name=f"enc{i}") for i in range(NBUF)]
            dmas = {}
            dmas[0] = nc.gpsimd.dma_start(
                out=enc_t[0][:], in_=enc_ext[0:PT, :, :]
            )
            bcast = nc.gpsimd.dma_start(
                out=xpb[:], in_=xp_dram[:].partition_broadcast(128)
            )
            add_dep_helper(bcast.ins, dmas[0].ins, sync=False)
            prev_d = bcast
            for k in range(1, NBUF):
                dmas[k] = nc.gpsimd.dma_start(
                    out=enc_t[k][:], in_=enc_ext[k * PT : (k + 1) * PT, :, :]
                )
                add_dep_helper(dmas[k].ins, prev_d.ins, sync=False)
                prev_d = dmas[k]

            # ---- persistent buffers for the t-tile loop -------------------
            # 2 rotating prod buffers; the jes/j2 observers carry all DMA
            # semaphores, so each multiply's only wait is its posted-write
            # hazard.
            prod = [sb.tile([PT, NB, E], F16, name=f"prod{i}") for i in range(2)]
            S_all = sb.tile([PT, NT, NB], F32)
            pT_all = sb.tile([NB, NT, PT], F16)
            pm_t = [sb.tile([PT, NB, NB], F16, name=f"pm{i}") for i in range(4)]
            for t in pm_t:
                nc.vector.memset(t[:], 0.0)  # off-diagonals stay 0 forever
            junk_ss = sb.tile([NB, NT], F32)
            dummy_all = sb.tile([PT, NT, NA], F32)  # write-once accum dummies
            jw = sb.tile([1, 8], F32)
            jns = sb.tile([NB, NT], F32)
            jha = sb.tile([NB, 2], F32)
            jese = sb.tile([1, NT], F32)
            jes2 = sb.tile([1, 1], F32)

            NEGM = sb.tile([NB, NT], F32)   # -m_k per (b, k)
            L_all = sb.tile([NB, NT], F32)  # l_k per (b, k)
            c_store = sb.tile([NB, NT, E], F16)

            # ---- software-pipelined t-tile loop --------------------------
            hist = {}
            tts = {}
            cstore_q = []  # tiles whose ctx_ps copy is still pending

            def emit_tail(k):
                eth = enc_t[k % NBUF]
                ctx_ps = ctx_pair[k % 2]
                jst = None
                if k == 0:
                    # PE observes tile 0's ScalarE-written S columns so the
                    # first transpose carries only the DVE wait
                    jst = nc.tensor.matmul(
                        obs_ps[:, 0:NA],
                        lhsT=S_all[:, 0, 0:1], rhs=S_all[:, 0, 0:NA],
                        start=True, stop=True,
                    )
                st_inst = nc.tensor.transpose(sT_ps[:], S_all[:, k, :], ident[0:PT, 0:PT])
                if jst is not None:
                    add_dep_helper(st_inst.ins, jst.ins, sync=False)
                if cstore_q:
                    # the pending c_store copy must land before this tile's
                    # ctx matmuls reuse ctx_ps; syncing the S transpose on it
                    # both orders the streams and keeps jmm at one wait
                    add_dep_helper(st_inst.ins, cstore_q[-1].ins, sync=True)
                if k >= 1:
                    # DVE observes exp(k-1) (the last sT_ps reader) so the
                    # NEGM reduce carries only the PE wait
                    nc.vector.tensor_copy(jns[:, k : k + 1], pT_all[:, k - 1, 0:1])
                nc.vector.tensor_reduce(
                    out=NEGM[:, k : k + 1], in_=sT_ps[:], axis=AX.X, op=ALU.max,
                    negate=True,
                )
                nc.scalar.activation(junk_ss[:, k : k + 1], sT_ps[:, 0:1], AF.Copy)
                nc.scalar.activation(
                    pT_all[:, k, :], sT_ps[:], AF.Exp,
                    bias=NEGM[:, k : k + 1], scale=1.0,
                    accum_out=L_all[:, k : k + 1],
                )
                nc.tensor.transpose(p_ps[:], pT_all[:, k, :], identH[0:NB, 0:NB])
                pm = pm_t[k % 4]
                nc.vector.tensor_copy(
                    pm[:, :, :].rearrange("p a b -> p (a b)")[:, :: NB + 1],
                    p_ps[:],
                )
                # PE observes the enc DMA(s) before the ctx matmuls use it
                jmm = nc.tensor.matmul(
                    ctx_ps[0:1, 0:16],
                    lhsT=eth[:, 0, 0:1], rhs=eth[:, 0, 0:16],
                    start=True, stop=True,
                )
                add_dep_helper(jmm.ins, st_inst.ins, sync=False)
                if isinstance(dmas[k], tuple):
                    jmm2 = nc.tensor.matmul(
                        ctx_ps[0:1, 0:16],
                        lhsT=eth[:, NTT2, 0:1], rhs=eth[:, NTT2, 0:16],
                        start=True, stop=True,
                    )
                    add_dep_helper(jmm2.ins, jmm.ins, sync=False)
                last_mm = None
                for b in range(NB):
                    last_mm = nc.tensor.matmul(
                        ctx_ps[:],
                        lhsT=pm[:, b, :],
                        rhs=eth[:, b, :],
                        start=(b == 0),
                        stop=(b == NB - 1),
                    )
                hist[k % NBUF] = (last_mm, tts[k], dmas[k][0] if isinstance(dmas[k], tuple) else dmas[k])
                return last_mm

            def emit_cstore(k):
                inst = nc.scalar.activation(
                    c_store[:, k, :], ctx_pair[k % 2][:], AF.Copy
                )
                cstore_q.append(inst)
                return inst

            for k in range(NT):
                eth = enc_t[k % NBUF]
                pr = prod[k % 2]
                # Pre-absorb the buffer's reuse hazards on the Pool proc
                # with explicitly-synced nops, so the SWDGE DMA needs no
                # more than the allowed number of waits.
                if k >= NBUF:
                    nop = None
                    for dep in hist[k % NBUF]:
                        prev_nop = nop
                        nop = nc.gpsimd.engine_nop()
                        add_dep_helper(nop.ins, dep.ins, sync=True)
                        if prev_nop is not None:
                            add_dep_helper(nop.ins, prev_nop.ins, sync=False)
                    if k == NT - 1:
                        # The final tile arrives as two batch-halves so its
                        # fp16 multiplies overlap the second half's transfer.
                        da = nc.gpsimd.dma_start(
                            out=eth[:, 0:NTT2, :],
                            in_=enc_ext[k * PT : (k + 1) * PT, 0:NTT2, :],
                        )
                        add_dep_helper(da.ins, nop.ins, sync=False)
                        db = nc.gpsimd.dma_start(
                            out=eth[:, NTT2:NB, :],
                            in_=enc_ext[k * PT : (k + 1) * PT, NTT2:NB, :],
                        )
                        add_dep_helper(db.ins, da.ins, sync=False)
                        dmas[k] = (da, db)
                    else:
                        dmas[k] = nc.gpsimd.dma_start(
                            out=eth[:], in_=enc_ext[k * PT : (k + 1) * PT, :, :]
                        )

                # --- DVE head -------------------------------------------
                jwl = None
                if k == 0:
                    # absorb the xpb waits for tt_a's batch range [0:NA):
                    # one observer for the ScalarE-copied odd batches, one
                    # for the DVE posted-write ack of the even batches
                    jw1 = nc.vector.tensor_copy(jw[0:1, 0:1], xpb[0:1, NA - 1, 0:1])
                    jwl = nc.vector.tensor_copy(jw[0:1, 1:2], xpb[0:1, NA - 2, 0:1])
                    add_dep_helper(jwl.ins, jw1.ins, sync=False)
                if k in (2, 3):
                    # ScalarE's reads of prod[k%2] (accums k-2) are not yet
                    # covered by the exp observer chain; absorb explicitly.
                    nc.vector.tensor_copy(
                        jha[:, k - 2 : k - 1], S_all[0:NB, k - 2, NA - 1 : NA]
                    )
                # A dedicated observer carries this tile's enc DMA wait so
                # the multiplies are free to carry their prod posted-write
                # self-waits.  Orders are pinned so the scheduler cannot
                # move the DMA wait onto a multiply.
                jes = nc.vector.tensor_copy(jese[0:1, k : k + 1], eth[0:1, 0, 0:1])
                if jwl is not None:
                    add_dep_helper(jes.ins, jwl.ins, sync=False)
                tt_a = nc.vector.tensor_tensor(
                    out=pr[:, 0:NA, :], in0=eth[:, 0:NA, :],
                    in1=xpb[:, 0:NA, :], op=ALU.mult,
                )
                add_dep_helper(tt_a.ins, jes.ins, sync=False)
                if k == NT - 1:
                    # split last tile: multiply the first-half batches, then
                    # observe the second-half DMA, then multiply the rest
                    tt_b1 = nc.vector.tensor_tensor(
                        out=pr[:, NA:NTT2, :], in0=eth[:, NA:NTT2, :],
                        in1=xpb[:, NA:NTT2, :], op=ALU.mult,
                    )
                    add_dep_helper(tt_b1.ins, tt_a.ins, sync=False)
                    j2 = nc.vector.tensor_copy(jes2[:], eth[0:1, NB - 1, 0:1])
                    add_dep_helper(j2.ins, tt_b1.ins, sync=False)
                    tt_b = nc.vector.tensor_tensor(
                        out=pr[:, NTT2:NB, :], in0=eth[:, NTT2:NB, :],
                        in1=xpb[:, NTT2:NB, :], op=ALU.mult,
                    )
                    add_dep_helper(tt_b.ins, j2.ins, sync=False)
                else:
                    tt_b = nc.vector.tensor_tensor(
                        out=pr[:, NA:NB, :], in0=eth[:, NA:NB, :],
                        in1=xpb[:, NA:NB, :], op=ALU.mult,
                    )
                    add_dep_helper(tt_b.ins, tt_a.ins, sync=False)
                # one batched reduce produces the DVE half of the S column
                red = nc.vector.tensor_reduce(
                    out=S_all[:, k, NA:NB], in_=pr[:, NA:NB, :], axis=AX.X,
                    op=ALU.add,
                )
                add_dep_helper(red.ins, tt_b.ins, sync=False)
                tts[k] = red

                # --- ScalarE head: activation-accumulate batches 0:NA ----
                for i, b in enumerate(range(0, NA)):
                    nc.scalar.activation(
                        dummy_all[:, k, i : i + 1].broadcast_to((PT, E)),
                        pr[:, b, :],
                        AF.Copy,
                        accum_out=S_all[:, k, b : b + 1],
                    )
                    if i == 0 and k >= LAG + 1:
                        emit_cstore(k - LAG - 1)

                if k >= LAG:
                    emit_tail(k - LAG)

            emit_tail(NT - 1)
            emit_cstore(NT - 2)

            # ---- final combine across tiles -------------------------------
            # Weight math on DVE, ordered after c_store(14) so the chain
            # ACT waits for tiles <= 14 all elide.  The 16-term weighted
            # sum runs as 2 DVE chains (tiles 0..7) + 2 GpSimd chains
            # (8..14), interleaved so posted-write self-waits hide, with
            # tile 15's term appended on DVE after its c_store lands.
            negM = sb.tile([NB, 1], F32)
            nc.vector.tensor_reduce(out=negM[:], in_=NEGM[:], axis=AX.X, op=ALU.min)
            alpha = sb.tile([NB, NT], F32)
            ainst = nc.scalar.activation(
                alpha[:], NEGM[:], AF.Exp, bias=negM[:], scale=-1.0
            )
            add_dep_helper(ainst.ins, cstore_q[-1].ins, sync=False)
            prodw = sb.tile([NB, NT], F32)
            nc.vector.tensor_tensor(out=prodw[:], in0=alpha[:], in1=L_all[:],
                                    op=ALU.mult)
            Lsum = sb.tile([NB, 1], F32)
            nc.vector.tensor_reduce(out=Lsum[:], in_=prodw[:], axis=AX.X, op=ALU.add)
            rL = sb.tile([NB, 1], F32)
            nc.vector.reciprocal(rL[:], Lsum[:])
            w = sb.tile([NB, NT], F32)
            nc.vector.tensor_scalar_mul(w[:], alpha[:], rL[:])

            # Weighted sum of the 16 stored contexts on DVE as TWO
            # interleaved accumulation chains, so each op's posted-write
            # self-wait hides under the other chain's execution.
            acc = out_tile[:, E : 2 * E]
            acc_b = sb.tile([NB, E], F32)
            nc.vector.tensor_copy(jw[0:1, 4:5], w[0:1, 0:1])  # absorb w's self-wait
            nc.vector.tensor_scalar_mul(acc, c_store[:, 0, :], w[:, 0:1])
            h = NT // 2
            nc.vector.tensor_scalar_mul(acc_b[:], c_store[:, h, :], w[:, h : h + 1])
            for k in range(1, h):
                nc.vector.scalar_tensor_tensor(
                    out=acc, in0=c_store[:, k, :], scalar=w[:, k : k + 1], in1=acc,
                    op0=ALU.mult, op1=ALU.add,
                )
                if h + k < NT - 1:
                    nc.vector.scalar_tensor_tensor(
                        out=acc_b[:], in0=c_store[:, h + k, :],
                        scalar=w[:, h + k : h + k + 1], in1=acc_b[:],
                        op0=ALU.mult, op1=ALU.add,
                    )

            emit_cstore(NT - 1)
            # DVE observes the last c_store copy, then finishes the chain
            nc.vector.tensor_copy(jw[0:1, 5:6], c_store[0:1, NT - 1, 0:1])
            nc.vector.scalar_tensor_tensor(
                out=acc_b[:], in0=c_store[:, NT - 1, :],
                scalar=w[:, NT - 1 : NT], in1=acc_b[:],
                op0=ALU.mult, op1=ALU.add,
            )
            nc.vector.tensor_tensor(out=acc, in0=acc, in1=acc_b[:], op=ALU.add)

            nc.sync.dma_start(out=out_ext[:, E : 2 * E], in_=acc)

    # Raw Bass does not lower InstISA subclasses (tensor_tensor_reduce);
    # without this the NEFF compiler sees empty .instr -> "ISA wrong length".
    mybir.codegen_inst_isa_subclasses(nc)
    return nc


_NC_CACHE: bass.Bass | None = None


def _get_nc() -> bass.Bass:
    global _NC_CACHE
    if _NC_CACHE is None:
        _NC_CACHE = build_nc()
    return _NC_CACHE


def make_in_maps(inputs: dict) -> list[dict]:
    x = np.asarray(inputs["x"], dtype=np.float32)
    enc = np.asarray(inputs["encoder_states"], dtype=np.float32)
    W = np.asarray(inputs["W"], dtype=np.float32)
    bias = np.asarray(inputs["b"], dtype=np.float32)

    # partition-major swizzle: element (p, c*X + j) = src[c*128 + p, j],
    # so each SBUF partition line is one contiguous DMA descriptor.
    WT16 = np.ascontiguousarray(
        W.T.astype(np.float16).reshape(NC_D, 128, E).transpose(1, 0, 2).reshape(
            128, NC_D * E
        )
    )
    b_rep = np.ascontiguousarray(np.broadcast_to(bias[None, :], (NB, E))).astype(
        np.float32
    )
    in_maps = []
    for i in range(NCORES):
        sl = slice(i * NB, (i + 1) * NB)
        xT16 = np.ascontiguousarray(
            x[sl].T.astype(np.float16)
            .reshape(NC_D, 128, NB).transpose(1, 0, 2).reshape(128, NC_D * NB)
        )
        in_maps.append(
            {
                "xT": xT16,
                "WT": WT16,
                "bias": b_rep,
                "enc": np.ascontiguousarray(enc[:, sl, :]),
            }
        )
    return in_maps


def run(inputs: dict, trace: bool = False, tmpdir: str | None = None):
    """Returns (full_output [B, 2E] f32, exec_time_ns or None)."""
    nc = _get_nc()
    in_maps = make_in_maps(inputs)
    res = run_bass_kernel_spmd(
        nc, in_maps, core_ids=list(range(NCORES)), trace=trace, tmpdir=tmpdir
    )
    out = np.concatenate([res.results[i]["out"] for i in range(NCORES)], axis=0)
    return out.astype(np.float32), res.exec_time_ns


def kernel(**inputs) -> np.ndarray:
    out, _ = run(inputs, trace=False)
    return out
